# revision 4
# baseline (speedup 1.0000x reference)
"""Grouped GEMM (MoE routing) kernel for 8 Trainium2 NeuronCores.

out[off_g : off_g + size_g] = A[off_g : off_g + size_g] @ B[g]   for g in 0..63
A: [524288, 256] f32, B: [64, 256, 256] f32, groups are contiguous row ranges.

Strategy (hardcoded, from the sharding hint "expert-parallel / data-parallel"):
  - Sort groups by tile count (128-row tiles), snake-assign one group per
    (slot, core): slot i takes the groups ranked [8i, 8i+8) — one per core.
    Every core then runs an IDENTICAL static schedule of sum(m_i) tiles,
    where m_i = max tile count in octile i (shorter groups zero-padded).
  - Host packs each core's 8 groups back to back (padded) and pre-transposes
    to AT_core [256, T*128] so the contraction dim is the SBUF partition dim.
  - Device: per-core B (8 experts, 2 MB) stays resident in SBUF; A streams
    in W-tile blocks; per 128-row tile: 2 accumulating matmuls (K=256 split
    across two 128-partition chunks), DVE copy PSUM->SBUF, batched out DMA.
"""

import os
import numpy as np

NCORES = 8
TILE = 128
K = 256
N = 256

# matmul operand dtype on device: "float32" (exact) or "float32r" (fast).
MM_DTYPE = os.environ.get("BASS_GG_DTYPE", "float32r")
W_TILES = int(os.environ.get("BASS_GG_W", "16"))  # tiles per A/out block

LAST_EXEC_NS = None  # set when BASS_GG_TRACE=1
LAST_RESULT = None

_prog_cache = {}


def _schedule(sizes):
    """sizes -> (slots [nslot, NCORES] group ids, m [nslot] tile budgets)."""
    sizes = np.asarray(sizes, dtype=np.int64)
    g = sizes.shape[0]
    pad_groups = (-g) % NCORES
    if pad_groups:
        sizes = np.concatenate([sizes, np.zeros(pad_groups, np.int64)])
    ntiles = (sizes + TILE - 1) // TILE
    order = np.argsort(-ntiles, kind="stable")
    nslot = len(sizes) // NCORES
    slots = order.reshape(nslot, NCORES)
    m = ntiles[slots[:, 0]].astype(np.int64)
    keep = m > 0
    return slots[keep], m[keep]


def _build_program(m_list, dtype_name, w_tiles):
    import concourse.tile as tile
    from concourse import bacc, mybir

    DT = getattr(mybir.dt, dtype_name)
    R = len(m_list)
    T = int(sum(m_list))

    nc = bacc.Bacc(
        "TRN2",
        target_bir_lowering=False,
        debug=False,
        enable_asserts=False,
        num_devices=NCORES,
    )
    AT = nc.dram_tensor("AT", [K, T * TILE], DT, kind="ExternalInput").ap()
    BW = nc.dram_tensor("BW", [R, 2, 128, N], DT, kind="ExternalInput").ap()
    OUT = nc.dram_tensor("OUT", [T * TILE, N], mybir.dt.float32, kind="ExternalOutput").ap()

    slot_of = []
    for i, mi in enumerate(m_list):
        slot_of += [i] * int(mi)

    with tile.TileContext(nc) as tc:
        with tc.tile_pool(name="bpool", bufs=1) as bpool, \
             tc.tile_pool(name="apool", bufs=3) as apool, \
             tc.tile_pool(name="opool", bufs=3) as opool, \
             tc.tile_pool(name="psum", bufs=8, space="PSUM") as pspool:
            b_sb = bpool.tile([128, R, 2, N], DT)
            nc.sync.dma_start(out=b_sb, in_=BW.rearrange("r j p n -> p r j n"))
            OUTv = OUT.rearrange("(t p) n -> p t n", p=TILE)
            nblk = (T + w_tiles - 1) // w_tiles
            for blk in range(nblk):
                t0 = blk * w_tiles
                w = min(w_tiles, T - t0)
                a0 = apool.tile([128, w_tiles * TILE], DT, tag="a0")
                a1 = apool.tile([128, w_tiles * TILE], DT, tag="a1")
                nc.sync.dma_start(
                    out=a0[:, : w * TILE], in_=AT[0:128, t0 * TILE : (t0 + w) * TILE]
                )
                nc.sync.dma_start(
                    out=a1[:, : w * TILE], in_=AT[128:256, t0 * TILE : (t0 + w) * TILE]
                )
                ob = opool.tile([128, w_tiles, N], mybir.dt.float32, tag="ob")
                for t in range(w):
                    s = slot_of[t0 + t]
                    ps = pspool.tile([128, N], mybir.dt.float32)
                    nc.tensor.matmul(
                        ps,
                        lhsT=a0[:, t * TILE : (t + 1) * TILE],
                        rhs=b_sb[:, s, 0, :],
                        start=True,
                        stop=False,
                    )
                    nc.tensor.matmul(
                        ps,
                        lhsT=a1[:, t * TILE : (t + 1) * TILE],
                        rhs=b_sb[:, s, 1, :],
                        start=False,
                        stop=True,
                    )
                    nc.vector.tensor_copy(out=ob[:, t, :], in_=ps)
                nc.scalar.dma_start(out=OUTv[:, t0 : t0 + w, :], in_=ob[:, :w, :])
    nc.compile()
    return nc


def _get_program(m_key, dtype_name, w_tiles):
    key = (m_key, dtype_name, w_tiles)
    if key not in _prog_cache:
        _prog_cache[key] = _build_program(list(m_key), dtype_name, w_tiles)
    return _prog_cache[key]


def kernel(A, B, batch_sizes, batch_offsets, batch_padded_offsets):
    global LAST_EXEC_NS, LAST_RESULT
    from concourse.bass_utils import run_bass_kernel_spmd

    A = np.asarray(A, dtype=np.float32)
    B = np.asarray(B, dtype=np.float32)
    sizes = np.asarray(batch_sizes, dtype=np.int64)
    offsets = np.asarray(batch_offsets, dtype=np.int64)

    M = A.shape[0]
    slots, m = _schedule(sizes)
    T = int(m.sum())
    starts = np.concatenate([[0], np.cumsum(m)[:-1]])  # slot start, in tiles

    nc = _get_program(tuple(int(x) for x in m), MM_DTYPE, W_TILES)

    in_maps = []
    for c in range(NCORES):
        at = np.zeros((K, T * TILE), dtype=np.float32)
        bw = np.zeros((len(m), 2, 128, N), dtype=np.float32)
        for i in range(len(m)):
            g = int(slots[i, c])
            off, sz = int(offsets[g]), int(sizes[g])
            dst = int(starts[i]) * TILE
            if sz > 0:
                at[:, dst : dst + sz] = A[off : off + sz].T
            bw[i] = B[g].reshape(2, 128, N)
        in_maps.append({"AT": at, "BW": bw})

    trace = bool(int(os.environ.get("BASS_GG_TRACE", "0")))
    res = run_bass_kernel_spmd(
        nc, in_maps, core_ids=list(range(NCORES)), trace=trace
    )
    LAST_EXEC_NS = res.exec_time_ns
    LAST_RESULT = res

    out = np.zeros((M, N), dtype=np.float32)
    for c in range(NCORES):
        oc = res.results[c]["OUT"]
        for i in range(len(m)):
            g = int(slots[i, c])
            off, sz = int(offsets[g]), int(sizes[g])
            src = int(starts[i]) * TILE
            if sz > 0:
                out[off : off + sz] += oc[src : src + sz]
    return out



# revision 10
# speedup vs baseline: 1.9347x; 1.9347x over previous
"""Grouped GEMM (MoE routing) kernel for 8 Trainium2 NeuronCores.

out[off_g : off_g + size_g] = A[off_g : off_g + size_g] @ B[g]   for g in 0..63
A: [524288, 256] f32, B: [64, 256, 256] f32, groups are contiguous row ranges.

Strategy (hardcoded, from the sharding hint "expert-parallel / data-parallel"):
  - Sort groups by tile count (128-row tiles), snake-assign one group per
    (slot, core): slot i takes the groups ranked [8i, 8i+8) — one per core.
    Every core then runs an IDENTICAL static schedule of sum(m_i) tiles,
    where m_i = max tile count in octile i (shorter groups zero-padded).
  - All device traffic is bf16 (PSUM accumulation stays fp32): halves both
    HBM bytes and tensor-engine passes vs fp32r.
  - Host packs each core's A into ATP [128, T, 2, 128] (k-partition-major,
    pre-transposed) and B into BWP [128, R, 2, N]; output comes back as
    OUT [128, T, N] bf16 and is de-interleaved host-side.
  - Device: per-core B stays resident in SBUF; per W-tile block: one in-DMA
    (16KB/partition runs), per 128-row tile 2 accumulating matmuls (K=256
    split over two 128-partition chunks), PSUM->SBUF convert-copy spread
    over vector/scalar/gpsimd, one out-DMA per block.
"""

import os
import numpy as np
import ml_dtypes

BF16 = ml_dtypes.bfloat16

NCORES = 8
TILE = 128
K = 256
N = 256

MM_DTYPE = os.environ.get("BASS_GG_DTYPE", "bfloat16")
W_TILES = int(os.environ.get("BASS_GG_W", "32"))  # tiles per A/out block
COPY_ENGINES = os.environ.get("BASS_GG_COPY", "vector,scalar").split(",")

LAST_EXEC_NS = None  # set when BASS_GG_TRACE=1
LAST_RESULT = None

_prog_cache = {}


def _schedule(sizes):
    """sizes -> (slots [nslot, NCORES] group ids, m [nslot] tile budgets)."""
    sizes = np.asarray(sizes, dtype=np.int64)
    g = sizes.shape[0]
    pad_groups = (-g) % NCORES
    if pad_groups:
        sizes = np.concatenate([sizes, np.zeros(pad_groups, np.int64)])
    ntiles = (sizes + TILE - 1) // TILE
    order = np.argsort(-ntiles, kind="stable")
    nslot = len(sizes) // NCORES
    slots = order.reshape(nslot, NCORES)
    m = ntiles[slots[:, 0]].astype(np.int64)
    keep = m > 0
    return slots[keep], m[keep]


def _build_program(m_list, dtype_name, w_tiles):
    import concourse.tile as tile
    from concourse import bacc, mybir

    DT = getattr(mybir.dt, dtype_name)
    R = len(m_list)
    T = int(sum(m_list))
    slot_of = []
    for i, mi in enumerate(m_list):
        slot_of += [i] * int(mi)

    nc = bacc.Bacc(
        "TRN2",
        target_bir_lowering=False,
        debug=False,
        enable_asserts=False,
        num_devices=NCORES,
    )
    ATP = nc.dram_tensor("ATP", [128, T, 2, TILE], DT, kind="ExternalInput").ap()
    BWP = nc.dram_tensor("BWP", [128, R, 2, N], DT, kind="ExternalInput").ap()
    OUT = nc.dram_tensor("OUT", [128, T, N], DT, kind="ExternalOutput").ap()

    copy_engines = [e.strip() for e in COPY_ENGINES if e.strip()]

    with tile.TileContext(nc) as tc:
        with tc.tile_pool(name="bpool", bufs=1) as bpool, \
             tc.tile_pool(name="apool", bufs=3) as apool, \
             tc.tile_pool(name="opool", bufs=3) as opool, \
             tc.tile_pool(name="psum", bufs=8, space="PSUM") as pspool:
            b_sb = bpool.tile([128, R, 2, N], DT)
            nc.sync.dma_start(out=b_sb, in_=BWP)
            nblk = (T + w_tiles - 1) // w_tiles
            for blk in range(nblk):
                t0 = blk * w_tiles
                w = min(w_tiles, T - t0)
                a = apool.tile([128, w_tiles, 2, TILE], DT, tag="a")
                nc.sync.dma_start(out=a[:, :w], in_=ATP[:, t0 : t0 + w])
                ob = opool.tile([128, w_tiles, N], DT, tag="ob")
                for t in range(w):
                    s = slot_of[t0 + t]
                    ps = pspool.tile([128, N], mybir.dt.float32)
                    nc.tensor.matmul(
                        ps,
                        lhsT=a[:, t, 0, :],
                        rhs=b_sb[:, s, 0, :],
                        start=True,
                        stop=False,
                    )
                    nc.tensor.matmul(
                        ps,
                        lhsT=a[:, t, 1, :],
                        rhs=b_sb[:, s, 1, :],
                        start=False,
                        stop=True,
                    )
                    eng = getattr(nc, copy_engines[t % len(copy_engines)])
                    if copy_engines[t % len(copy_engines)] == "scalar":
                        eng.copy(out=ob[:, t, :], in_=ps)
                    else:
                        eng.tensor_copy(out=ob[:, t, :], in_=ps)
                nc.sync.dma_start(out=OUT[:, t0 : t0 + w, :], in_=ob[:, :w])
    nc.compile()
    return nc


def _get_program(m_key, dtype_name, w_tiles):
    key = (m_key, dtype_name, w_tiles)
    if key not in _prog_cache:
        _prog_cache[key] = _build_program(list(m_key), dtype_name, w_tiles)
    return _prog_cache[key]


def kernel(A, B, batch_sizes, batch_offsets, batch_padded_offsets):
    global LAST_EXEC_NS, LAST_RESULT
    from concourse.bass_utils import run_bass_kernel_spmd

    A = np.asarray(A, dtype=np.float32)
    B = np.asarray(B, dtype=np.float32)
    sizes = np.asarray(batch_sizes, dtype=np.int64)
    offsets = np.asarray(batch_offsets, dtype=np.int64)

    M = A.shape[0]
    slots, m = _schedule(sizes)
    T = int(m.sum())
    R = len(m)
    starts = np.concatenate([[0], np.cumsum(m)[:-1]])  # slot start, in tiles

    nc = _get_program(tuple(int(x) for x in m), MM_DTYPE, W_TILES)

    A16 = A.astype(BF16)
    B16 = B.astype(BF16)

    in_maps = []
    for c in range(NCORES):
        atp = np.zeros((128, T, 2, TILE), dtype=BF16)
        bwp = np.zeros((128, R, 2, N), dtype=BF16)
        for i in range(R):
            g = int(slots[i, c])
            off, sz = int(offsets[g]), int(sizes[g])
            s0, mi = int(starts[i]), int(m[i])
            if sz > 0:
                ag = np.zeros((mi * TILE, K), dtype=BF16)
                ag[:sz] = A16[off : off + sz]
                atp[:, s0 : s0 + mi] = ag.reshape(mi, TILE, 2, 128).transpose(
                    3, 0, 2, 1
                )
            bwp[:, i] = B16[g].reshape(2, 128, N).transpose(1, 0, 2)
        in_maps.append({"ATP": atp, "BWP": bwp})

    trace = bool(int(os.environ.get("BASS_GG_TRACE", "0")))
    res = run_bass_kernel_spmd(
        nc, in_maps, core_ids=list(range(NCORES)), trace=trace
    )
    LAST_EXEC_NS = res.exec_time_ns
    LAST_RESULT = res

    out = np.zeros((M, N), dtype=np.float32)
    for c in range(NCORES):
        oc = res.results[c]["OUT"]
        for i in range(R):
            g = int(slots[i, c])
            off, sz = int(offsets[g]), int(sizes[g])
            s0, mi = int(starts[i]), int(m[i])
            if sz > 0:
                blk = oc[:, s0 : s0 + mi, :].transpose(1, 0, 2).reshape(
                    mi * TILE, N
                )
                out[off : off + sz] = blk[:sz].astype(np.float32)
    return out
